# revision 8
# baseline (speedup 1.0000x reference)
"""EvolveGCN-H single-forward Bass kernel for Trainium2.

Strategy: the graph is tiny (129 nodes), so the full forward runs on every
core (replicated SPMD, no collectives); the host only re-lays-out inputs.

Device-side computation (per core):
  1. score  = tanh((x @ p) / ||p||)      -- PE matmuls + Sqrt/recip + Tanh
  2. rank_i = #{j: s_j > s_i} + #{j<i: s_j == s_i}  (== stable argsort-desc)
     via a broadcast comparison matrix on the vector engine; the raw
     (pre-tanh) scores are used for comparisons (tanh is monotonic).
  3. x_tildeT = (x * score)^T permuted with a one-hot matrix P^T[i,r] =
     (rank_i == r) via PE matmul.
  4. GRU: gi/gh matmuls with the bias folded in as an extra contraction
     row; gates on ACT/DVE; W = cand + z*(W0 - cand).
  5. GCN aggregation: dense normalized adjacency built from the edge list
     with one-hot matmuls:  ArawT[s,t] = sum_e (row_e==s) * ew_e*(col_e==t),
     accumulated over 34 edge tiles in PSUM.  deg = colsum/rowsum of ArawT
     (via ones-matmuls), dis = 1/sqrt(deg), out^T = (dis*xW)^T @ ArawT
     scaled by dis on the free axis.
  6. ELU(v) = relu(v) + exp(min(v,0)) - 1, final linear with folded bias.

All shapes are hardcoded for N=IN=129, OUT=64, E=4096.
"""

import sys

import numpy as np

if "/opt/trn_rl_repo" not in sys.path:
    sys.path.insert(0, "/opt/trn_rl_repo")

N = 129          # nodes
IN = 129         # in_channels
OUT = 64         # out_channels
E = 4096         # edges
G = 3 * IN       # GRU gate width (387)
NE = E + N       # edges incl. self loops (4225)
ETILES = (NE + 127) // 128   # 34
P = 128

_CACHE = {}


def _build():
    from concourse import bacc, mybir
    from concourse.masks import make_identity
    from concourse.tile import TileContext

    f32 = mybir.dt.float32
    AF = mybir.ActivationFunctionType
    OP = mybir.AluOpType

    nc = bacc.Bacc(None)

    def din(name, shape):
        return nc.dram_tensor(name, shape, f32, kind="ExternalInput")

    x_n = din("x_n", [N, IN])        # x natural
    x_t = din("x_t", [IN, N])        # x transposed
    w0_n = din("w0_n", [IN, IN])     # W0 natural
    w0_t = din("w0_t", [IN, IN])     # W0 transposed
    wih_t = din("wih_t", [IN, G])    # w_ih.T
    whh_t = din("whh_t", [IN, G])    # w_hh.T
    linw_t = din("linw_t", [IN, OUT])
    bih_r = din("bih_r", [1, G])
    bhh_r = din("bhh_r", [1, G])
    linb_r = din("linb_r", [1, OUT])
    cbias_c = din("cbias_c", [N, 1])
    p_c = din("p_c", [IN, 1])
    iota_t = din("iota_t", [P, N])   # iota along free dim, same per partition
    iota_c = din("iota_c", [N, 1])   # iota along partitions
    ones_c = din("ones_c", [N, 1])
    ones_r = din("ones_r", [1, P])
    edges = din("edges", [P, ETILES * 3])  # packed (row, col, ew) per e-tile

    out_d = nc.dram_tensor("out", [N, OUT], f32, kind="ExternalOutput")

    with TileContext(nc) as tc:
        with (
            tc.tile_pool(name="cons", bufs=1) as cons,
            tc.tile_pool(name="work", bufs=1) as work,
            tc.tile_pool(name="oh", bufs=6) as ohp,
            tc.tile_pool(name="acc", bufs=2, space="PSUM") as acc,
            tc.tile_pool(name="ps", bufs=6, space="PSUM") as ps,
        )        :
            # ---- input loads (main [128, .] + tail [1, .] for 129-row tensors)
            def load2(dram, w, tag):
                m = cons.tile([P, w], f32, tag=tag + "_m")
                t = cons.tile([1, w], f32, tag=tag + "_t")
                nc.sync.dma_start(out=m[:], in_=dram[0:P, :])
                nc.sync.dma_start(out=t[:], in_=dram[P : P + 1, :])
                return m, t

        # NOTE: indentation level of the block above continues below.
            xn_m, xn_t = load2(x_n, IN, "xn")
            xt_m, xt_t = load2(x_t, N, "xt")
            w0n_m, w0n_t = load2(w0_n, IN, "w0n")
            w0t_m, w0t_t = load2(w0_t, IN, "w0t")
            wih_m, wih_tt = load2(wih_t, G, "wih")
            whh_m, whh_tt = load2(whh_t, G, "whh")
            lw_m, lw_t = load2(linw_t, OUT, "lw")
            pc_m, pc_t = load2(p_c, 1, "pc")
            cb_m, cb_t = load2(cbias_c, 1, "cb")
            ic_m, _ic_t = load2(iota_c, 1, "ic")
            oc_m, oc_t = load2(ones_c, 1, "oc")

            def load1(dram, pdim, w, tag):
                s = cons.tile([pdim, w], f32, tag=tag)
                nc.sync.dma_start(out=s[:], in_=dram[:])
                return s

            bih_s = load1(bih_r, 1, G, "bih")
            bhh_s = load1(bhh_r, 1, G, "bhh")
            lb_s = load1(linb_r, 1, OUT, "lb")
            io_s = load1(iota_t, P, N, "io")
            or_s = load1(ones_r, 1, P, "or")
            ed_s = load1(edges, P, ETILES * 3, "ed")

            ident = cons.tile([P, P], f32, tag="ident")
            make_identity(nc, ident[:])

            # ================= score (raw + tanh) =================
            # ||p||^2
            pn_ps = ps.tile([1, 1], f32, tag="ps")
            nc.tensor.matmul(out=pn_ps[:], lhsT=pc_m[:], rhs=pc_m[:], start=True, stop=False)
            nc.tensor.matmul(out=pn_ps[:], lhsT=pc_t[:], rhs=pc_t[:], start=False, stop=True)
            pn_s = work.tile([1, 1], f32, tag="pn")
            nc.scalar.activation(out=pn_s[:], in_=pn_ps[:], func=AF.Sqrt)
            invn = work.tile([1, 1], f32, tag="invn")
            nc.vector.reciprocal(out=invn[:], in_=pn_s[:])
            # broadcast 1/||p|| to 128 partitions
            invb_ps = ps.tile([P, 1], f32, tag="ps")
            nc.tensor.matmul(out=invb_ps[:], lhsT=or_s[:], rhs=invn[:], start=True, stop=True)
            invb = work.tile([P, 1], f32, tag="invb")
            nc.vector.tensor_copy(out=invb[:], in_=invb_ps[:])

            # raw scores (column layout), sraw[i] = x[i,:] @ p
            sraw_m_ps = ps.tile([P, 1], f32, tag="ps")
            nc.tensor.matmul(out=sraw_m_ps[:], lhsT=xt_m[:, 0:P], rhs=pc_m[:], start=True, stop=False)
            nc.tensor.matmul(out=sraw_m_ps[:], lhsT=xt_t[:, 0:P], rhs=pc_t[:], start=False, stop=True)
            sraw_t_ps = ps.tile([1, 1], f32, tag="ps")
            nc.tensor.matmul(out=sraw_t_ps[:], lhsT=xt_m[:, P : P + 1], rhs=pc_m[:], start=True, stop=False)
            nc.tensor.matmul(out=sraw_t_ps[:], lhsT=xt_t[:, P : P + 1], rhs=pc_t[:], start=False, stop=True)
            sraw_m = work.tile([P, 1], f32, tag="sraw_m")
            sraw_t = work.tile([1, 1], f32, tag="sraw_t")
            nc.vector.tensor_copy(out=sraw_m[:], in_=sraw_m_ps[:])
            nc.vector.tensor_copy(out=sraw_t[:], in_=sraw_t_ps[:])

            # tanh(sraw/||p||) in column layout (used to scale x rows)
            score_m = work.tile([P, 1], f32, tag="score_m")
            score_t = work.tile([1, 1], f32, tag="score_t")
            nc.scalar.activation(out=score_m[:], in_=sraw_m_ps[:], func=AF.Tanh, scale=invb[:])
            nc.scalar.activation(out=score_t[:], in_=sraw_t_ps[:], func=AF.Tanh, scale=invn[:])

            # raw scores as a row: transpose main block, copy tail element
            srT_ps = ps.tile([1, P], f32, tag="ps")
            nc.tensor.transpose(out=srT_ps[:], in_=sraw_m[:], identity=ident[:])
            srow = work.tile([1, N], f32, tag="srow")
            nc.vector.tensor_copy(out=srow[:, 0:P], in_=srT_ps[:])
            nc.vector.tensor_copy(out=srow[:, P : P + 1], in_=sraw_t[:])
            # broadcast raw-score row to all partitions
            srb_ps = ps.tile([P, N], f32, tag="ps")
            nc.tensor.matmul(out=srb_ps[:], lhsT=or_s[:], rhs=srow[:], start=True, stop=True)

            # ================= ranks =================
            # rank_i = sum_j (s_j > s_i) + (j < i)*(s_j == s_i)
            gt_m = work.tile([P, N], f32, tag="gt_m")
            nc.vector.tensor_tensor(out=gt_m[:], in0=srb_ps[:], in1=sraw_m[:].to_broadcast([P, N]), op=OP.is_gt)
            eq_m = work.tile([P, N], f32, tag="eq_m")
            nc.vector.tensor_tensor(out=eq_m[:], in0=srb_ps[:], in1=sraw_m[:].to_broadcast([P, N]), op=OP.is_equal)
            lt_m = work.tile([P, N], f32, tag="lt_m")
            nc.vector.tensor_tensor(out=lt_m[:], in0=io_s[:], in1=ic_m[:].to_broadcast([P, N]), op=OP.is_lt)
            meq_m = work.tile([P, N], f32, tag="meq_m")
            nc.vector.tensor_tensor(out=meq_m[:], in0=eq_m[:], in1=lt_m[:], op=OP.mult)
            cst_m = work.tile([P, N], f32, tag="cst_m")
            nc.vector.tensor_tensor(out=cst_m[:], in0=gt_m[:], in1=meq_m[:], op=OP.add)
            rank_m = work.tile([P, 1], f32, tag="rank_m")
            nc.vector.tensor_reduce(out=rank_m[:], in_=cst_m[:], axis=mybir.AxisListType.X, op=OP.add)
            # tail node (i = 128)
            gt_t = work.tile([1, N], f32, tag="gt_t")
            nc.vector.tensor_tensor(out=gt_t[:], in0=srow[:], in1=sraw_t[:].to_broadcast([1, N]), op=OP.is_gt)
            eq_t = work.tile([1, N], f32, tag="eq_t")
            nc.vector.tensor_tensor(out=eq_t[:], in0=srow[:], in1=sraw_t[:].to_broadcast([1, N]), op=OP.is_equal)
            lt_t = work.tile([1, N], f32, tag="lt_t")
            nc.vector.tensor_scalar(out=lt_t[:], in0=io_s[0:1, :], scalar1=float(P), scalar2=None, op0=OP.is_lt)
            meq_t = work.tile([1, N], f32, tag="meq_t")
            nc.vector.tensor_tensor(out=meq_t[:], in0=eq_t[:], in1=lt_t[:], op=OP.mult)
            cst_t = work.tile([1, N], f32, tag="cst_t")
            nc.vector.tensor_tensor(out=cst_t[:], in0=gt_t[:], in1=meq_t[:], op=OP.add)
            rank_t = work.tile([1, 1], f32, tag="rank_t")
            nc.vector.tensor_reduce(out=rank_t[:], in_=cst_t[:], axis=mybir.AxisListType.X, op=OP.add)

            # one-hot permutation: PT[i, r] = (rank_i == r)
            pt_m = work.tile([P, N], f32, tag="pt_m")
            nc.vector.tensor_tensor(out=pt_m[:], in0=io_s[:], in1=rank_m[:].to_broadcast([P, N]), op=OP.is_equal)
            pt_t = work.tile([1, N], f32, tag="pt_t")
            nc.vector.tensor_tensor(out=pt_t[:], in0=io_s[0:1, :], in1=rank_t[:].to_broadcast([1, N]), op=OP.is_equal)

            # ================= x_tilde^T =================
            sx_m = work.tile([P, IN], f32, tag="sx_m")
            nc.vector.tensor_tensor(out=sx_m[:], in0=xn_m[:], in1=score_m[:].to_broadcast([P, IN]), op=OP.mult)
            sx_t = work.tile([1, IN], f32, tag="sx_t")
            nc.vector.tensor_tensor(out=sx_t[:], in0=xn_t[:], in1=score_t[:].to_broadcast([1, IN]), op=OP.mult)

            xtt_m_ps = ps.tile([P, N], f32, tag="ps")
            nc.tensor.matmul(out=xtt_m_ps[:], lhsT=sx_m[:, 0:P], rhs=pt_m[:], start=True, stop=False)
            nc.tensor.matmul(out=xtt_m_ps[:], lhsT=sx_t[:, 0:P], rhs=pt_t[:], start=False, stop=True)
            xtt_t_ps = ps.tile([1, N], f32, tag="ps")
            nc.tensor.matmul(out=xtt_t_ps[:], lhsT=sx_m[:, P : P + 1], rhs=pt_m[:], start=True, stop=False)
            nc.tensor.matmul(out=xtt_t_ps[:], lhsT=sx_t[:, P : P + 1], rhs=pt_t[:], start=False, stop=True)
            xtt_m = work.tile([P, N], f32, tag="xtt_m")
            xtt_t = work.tile([1, N], f32, tag="xtt_t")
            nc.vector.tensor_copy(out=xtt_m[:], in_=xtt_m_ps[:])
            nc.vector.tensor_copy(out=xtt_t[:], in_=xtt_t_ps[:])

            # ================= GRU gates =================
            gi_m_ps = ps.tile([P, G], f32, tag="ps")
            nc.tensor.matmul(out=gi_m_ps[:], lhsT=xtt_m[:, 0:P], rhs=wih_m[:], start=True, stop=False)
            nc.tensor.matmul(out=gi_m_ps[:], lhsT=xtt_t[:, 0:P], rhs=wih_tt[:], start=False, stop=False)
            nc.tensor.matmul(out=gi_m_ps[:], lhsT=or_s[:, 0:P], rhs=bih_s[:], start=False, stop=True)
            gi_t_ps = ps.tile([1, G], f32, tag="ps")
            nc.tensor.matmul(out=gi_t_ps[:], lhsT=xtt_m[:, P : P + 1], rhs=wih_m[:], start=True, stop=False)
            nc.tensor.matmul(out=gi_t_ps[:], lhsT=xtt_t[:, P : P + 1], rhs=wih_tt[:], start=False, stop=False)
            nc.tensor.matmul(out=gi_t_ps[:], lhsT=or_s[:, 0:1], rhs=bih_s[:], start=False, stop=True)

            gh_m_ps = ps.tile([P, G], f32, tag="ps")
            nc.tensor.matmul(out=gh_m_ps[:], lhsT=w0t_m[:, 0:P], rhs=whh_m[:], start=True, stop=False)
            nc.tensor.matmul(out=gh_m_ps[:], lhsT=w0t_t[:, 0:P], rhs=whh_tt[:], start=False, stop=False)
            nc.tensor.matmul(out=gh_m_ps[:], lhsT=or_s[:, 0:P], rhs=bhh_s[:], start=False, stop=True)
            gh_t_ps = ps.tile([1, G], f32, tag="ps")
            nc.tensor.matmul(out=gh_t_ps[:], lhsT=w0t_m[:, P : P + 1], rhs=whh_m[:], start=True, stop=False)
            nc.tensor.matmul(out=gh_t_ps[:], lhsT=w0t_t[:, P : P + 1], rhs=whh_tt[:], start=False, stop=False)
            nc.tensor.matmul(out=gh_t_ps[:], lhsT=or_s[:, 0:1], rhs=bhh_s[:], start=False, stop=True)

            # move gh to SBUF so elementwise ops read one PSUM + one SBUF
            gh_m = work.tile([P, G], f32, tag="gh_m")
            gh_t = work.tile([1, G], f32, tag="gh_t")
            nc.vector.tensor_copy(out=gh_m[:], in_=gh_m_ps[:])
            nc.vector.tensor_copy(out=gh_t[:], in_=gh_t_ps[:])

            def gru_chunk(pdim, gi_ps, gh_sb, w0_sb, tag):
                rp = work.tile([pdim, IN], f32, tag="rp" + tag)
                nc.vector.tensor_tensor(out=rp[:], in0=gi_ps[:, 0:IN], in1=gh_sb[:, 0:IN], op=OP.add)
                r = work.tile([pdim, IN], f32, tag="r" + tag)
                nc.scalar.activation(out=r[:], in_=rp[:], func=AF.Sigmoid)
                zp = work.tile([pdim, IN], f32, tag="zp" + tag)
                nc.vector.tensor_tensor(out=zp[:], in0=gi_ps[:, IN : 2 * IN], in1=gh_sb[:, IN : 2 * IN], op=OP.add)
                z = work.tile([pdim, IN], f32, tag="z" + tag)
                nc.scalar.activation(out=z[:], in_=zp[:], func=AF.Sigmoid)
                rh = work.tile([pdim, IN], f32, tag="rh" + tag)
                nc.vector.tensor_tensor(out=rh[:], in0=r[:], in1=gh_sb[:, 2 * IN : 3 * IN], op=OP.mult)
                cp = work.tile([pdim, IN], f32, tag="cp" + tag)
                nc.vector.tensor_tensor(out=cp[:], in0=gi_ps[:, 2 * IN : 3 * IN], in1=rh[:], op=OP.add)
                cand = work.tile([pdim, IN], f32, tag="cand" + tag)
                nc.scalar.activation(out=cand[:], in_=cp[:], func=AF.Tanh)
                d = work.tile([pdim, IN], f32, tag="d" + tag)
                nc.vector.tensor_tensor(out=d[:], in0=w0_sb[:], in1=cand[:], op=OP.subtract)
                zd = work.tile([pdim, IN], f32, tag="zd" + tag)
                nc.vector.tensor_tensor(out=zd[:], in0=z[:], in1=d[:], op=OP.mult)
                w = work.tile([pdim, IN], f32, tag="w" + tag)
                nc.vector.tensor_tensor(out=w[:], in0=cand[:], in1=zd[:], op=OP.add)
                return w

            w_m = gru_chunk(P, gi_m_ps, gh_m, w0n_m, "_m")
            w_t = gru_chunk(1, gi_t_ps, gh_t, w0n_t, "_t")

            # ================= adjacency (one-hot matmuls) =================
            araw_m_ps = acc.tile([P, N], f32, tag="acc")
            araw_t_ps = acc.tile([1, N], f32, tag="acc")
            for n in range(ETILES):
                roh = ohp.tile([P, N], f32, tag="roh")
                coh = ohp.tile([P, N], f32, tag="coh")
                nc.vector.tensor_tensor(
                    out=roh[:], in0=io_s[:],
                    in1=ed_s[:, 3 * n : 3 * n + 1].to_broadcast([P, N]), op=OP.is_equal,
                )
                ceq = ohp.tile([P, N], f32, tag="ceq")
                nc.vector.tensor_tensor(
                    out=ceq[:], in0=io_s[:],
                    in1=ed_s[:, 3 * n + 1 : 3 * n + 2].to_broadcast([P, N]), op=OP.is_equal,
                )
                nc.vector.tensor_tensor(
                    out=coh[:], in0=ceq[:],
                    in1=ed_s[:, 3 * n + 2 : 3 * n + 3].to_broadcast([P, N]), op=OP.mult,
                )
                first, last = n == 0, n == ETILES - 1
                nc.tensor.matmul(out=araw_m_ps[:], lhsT=roh[:, 0:P], rhs=coh[:], start=first, stop=last)
                nc.tensor.matmul(out=araw_t_ps[:], lhsT=roh[:, P : P + 1], rhs=coh[:], start=first, stop=last)
            araw_m = work.tile([P, N], f32, tag="araw_m")
            araw_t = work.tile([1, N], f32, tag="araw_t")
            nc.vector.tensor_copy(out=araw_m[:], in_=araw_m_ps[:])
            nc.vector.tensor_copy(out=araw_t[:], in_=araw_t_ps[:])

            # ================= degrees / dis =================
            degr_ps = ps.tile([1, N], f32, tag="ps")   # deg as a row (free axis)
            nc.tensor.matmul(out=degr_ps[:], lhsT=oc_m[:], rhs=araw_m[:], start=True, stop=False)
            nc.tensor.matmul(out=degr_ps[:], lhsT=oc_t[:], rhs=araw_t[:], start=False, stop=True)
            degc_m_ps = ps.tile([P, 1], f32, tag="ps")  # deg as a column (partitions)
            nc.tensor.matmul(out=degc_m_ps[:], lhsT=araw_m[:, 0:P], rhs=oc_m[:], start=True, stop=False)
            nc.tensor.matmul(out=degc_m_ps[:], lhsT=araw_t[:, 0:P], rhs=oc_t[:], start=False, stop=True)
            degc_t_ps = ps.tile([1, 1], f32, tag="ps")
            nc.tensor.matmul(out=degc_t_ps[:], lhsT=araw_m[:, P : P + 1], rhs=oc_m[:], start=True, stop=False)
            nc.tensor.matmul(out=degc_t_ps[:], lhsT=araw_t[:, P : P + 1], rhs=oc_t[:], start=False, stop=True)

            sdr = work.tile([1, N], f32, tag="sdr")
            nc.scalar.activation(out=sdr[:], in_=degr_ps[:], func=AF.Sqrt)
            disr = work.tile([1, N], f32, tag="disr")
            nc.vector.reciprocal(out=disr[:], in_=sdr[:])
            sdc_m = work.tile([P, 1], f32, tag="sdc_m")
            nc.scalar.activation(out=sdc_m[:], in_=degc_m_ps[:], func=AF.Sqrt)
            disc_m = work.tile([P, 1], f32, tag="disc_m")
            nc.vector.reciprocal(out=disc_m[:], in_=sdc_m[:])
            sdc_t = work.tile([1, 1], f32, tag="sdc_t")
            nc.scalar.activation(out=sdc_t[:], in_=degc_t_ps[:], func=AF.Sqrt)
            disc_t = work.tile([1, 1], f32, tag="disc_t")
            nc.vector.reciprocal(out=disc_t[:], in_=sdc_t[:])

            # dis row broadcast to all partitions (for free-axis scaling)
            disb_ps = ps.tile([P, N], f32, tag="ps")
            nc.tensor.matmul(out=disb_ps[:], lhsT=or_s[:], rhs=disr[:], start=True, stop=True)
            disb = work.tile([P, N], f32, tag="disb")
            nc.vector.tensor_copy(out=disb[:], in_=disb_ps[:])

            # ================= x @ W, source scaling =================
            xw_m_ps = ps.tile([P, IN], f32, tag="ps")
            nc.tensor.matmul(out=xw_m_ps[:], lhsT=xt_m[:, 0:P], rhs=w_m[:], start=True, stop=False)
            nc.tensor.matmul(out=xw_m_ps[:], lhsT=xt_t[:, 0:P], rhs=w_t[:], start=False, stop=True)
            xw_t_ps = ps.tile([1, IN], f32, tag="ps")
            nc.tensor.matmul(out=xw_t_ps[:], lhsT=xt_m[:, P : P + 1], rhs=w_m[:], start=True, stop=False)
            nc.tensor.matmul(out=xw_t_ps[:], lhsT=xt_t[:, P : P + 1], rhs=w_t[:], start=False, stop=True)
            y_m = work.tile([P, IN], f32, tag="y_m")
            nc.vector.tensor_tensor(out=y_m[:], in0=xw_m_ps[:], in1=disc_m[:].to_broadcast([P, IN]), op=OP.mult)
            y_t = work.tile([1, IN], f32, tag="y_t")
            nc.vector.tensor_tensor(out=y_t[:], in0=xw_t_ps[:], in1=disc_t[:].to_broadcast([1, IN]), op=OP.mult)

            # ================= aggregate: gcnT = y^T-contract with ArawT ====
            gcnT_m_ps = ps.tile([P, N], f32, tag="ps")
            nc.tensor.matmul(out=gcnT_m_ps[:], lhsT=y_m[:, 0:P], rhs=araw_m[:], start=True, stop=False)
            nc.tensor.matmul(out=gcnT_m_ps[:], lhsT=y_t[:, 0:P], rhs=araw_t[:], start=False, stop=True)
            gcnT_t_ps = ps.tile([1, N], f32, tag="ps")
            nc.tensor.matmul(out=gcnT_t_ps[:], lhsT=y_m[:, P : P + 1], rhs=araw_m[:], start=True, stop=False)
            nc.tensor.matmul(out=gcnT_t_ps[:], lhsT=y_t[:, P : P + 1], rhs=araw_t[:], start=False, stop=True)

            def elu_chunk(pdim, gcn_ps, dis_row, cbias, tag):
                v1 = work.tile([pdim, N], f32, tag="v1" + tag)
                nc.vector.tensor_tensor(out=v1[:], in0=gcn_ps[:], in1=dis_row[:], op=OP.mult)
                v2 = work.tile([pdim, N], f32, tag="v2" + tag)
                nc.vector.tensor_tensor(out=v2[:], in0=v1[:], in1=cbias[:].to_broadcast([pdim, N]), op=OP.add)
                m0 = work.tile([pdim, N], f32, tag="m0" + tag)
                nc.vector.tensor_scalar(out=m0[:], in0=v2[:], scalar1=0.0, scalar2=None, op0=OP.min)
                e0 = work.tile([pdim, N], f32, tag="e0" + tag)
                nc.scalar.activation(out=e0[:], in_=m0[:], func=AF.Exp)
                r0 = work.tile([pdim, N], f32, tag="r0" + tag)
                nc.scalar.activation(out=r0[:], in_=v2[:], func=AF.Relu)
                h1 = work.tile([pdim, N], f32, tag="h1" + tag)
                nc.vector.tensor_tensor(out=h1[:], in0=r0[:], in1=e0[:], op=OP.add)
                h2 = work.tile([pdim, N], f32, tag="h2" + tag)
                nc.vector.tensor_scalar(out=h2[:], in0=h1[:], scalar1=-1.0, scalar2=None, op0=OP.add)
                return h2

            hT_m = elu_chunk(P, gcnT_m_ps, disb, cb_m, "_m")    # [u, t] u=0..127
            hT_t = elu_chunk(1, gcnT_t_ps, disr, cb_t, "_t")    # [u, t] u=128

            # ================= final linear =================
            o_m_ps = ps.tile([P, OUT], f32, tag="ps")
            nc.tensor.matmul(out=o_m_ps[:], lhsT=hT_m[:, 0:P], rhs=lw_m[:], start=True, stop=False)
            nc.tensor.matmul(out=o_m_ps[:], lhsT=hT_t[:, 0:P], rhs=lw_t[:], start=False, stop=False)
            nc.tensor.matmul(out=o_m_ps[:], lhsT=or_s[:, 0:P], rhs=lb_s[:], start=False, stop=True)
            o_t_ps = ps.tile([1, OUT], f32, tag="ps")
            nc.tensor.matmul(out=o_t_ps[:], lhsT=hT_m[:, P : P + 1], rhs=lw_m[:], start=True, stop=False)
            nc.tensor.matmul(out=o_t_ps[:], lhsT=hT_t[:, P : P + 1], rhs=lw_t[:], start=False, stop=False)
            nc.tensor.matmul(out=o_t_ps[:], lhsT=or_s[:, 0:1], rhs=lb_s[:], start=False, stop=True)

            ob_m = work.tile([P, OUT], f32, tag="ob_m")
            nc.vector.tensor_copy(out=ob_m[:], in_=o_m_ps[:])
            ob_t = work.tile([1, OUT], f32, tag="ob_t")
            nc.vector.tensor_copy(out=ob_t[:], in_=o_t_ps[:])
            nc.sync.dma_start(out=out_d[0:P, :], in_=ob_m[:])
            nc.sync.dma_start(out=out_d[P : P + 1, :], in_=ob_t[:])

    nc.finalize()
    return nc


def _pack(inputs):
    f = np.float32
    x = np.ascontiguousarray(np.asarray(inputs["x"], f))
    ei = np.asarray(inputs["edge_index"]).astype(np.int64)
    ew = np.asarray(inputs["edge_weight"], f)
    pool_p = np.asarray(inputs["pool_p"], f).reshape(IN)
    W0 = np.asarray(inputs["W0"], f)
    w_ih = np.asarray(inputs["w_ih"], f)
    w_hh = np.asarray(inputs["w_hh"], f)
    b_ih = np.asarray(inputs["b_ih"], f).reshape(G)
    b_hh = np.asarray(inputs["b_hh"], f).reshape(G)
    conv_bias = np.asarray(inputs["conv_bias"], f).reshape(IN)
    lin_w = np.asarray(inputs["lin_w"], f)
    lin_b = np.asarray(inputs["lin_b"], f).reshape(OUT)

    loop = np.arange(N, dtype=np.int64)
    row_f = np.concatenate([ei[0], loop])
    col_f = np.concatenate([ei[1], loop])
    ew_f = np.concatenate([ew, np.ones(N, f)])
    pad = ETILES * P - NE
    row_f = np.concatenate([row_f, np.zeros(pad, np.int64)])
    col_f = np.concatenate([col_f, np.zeros(pad, np.int64)])
    ew_f = np.concatenate([ew_f, np.zeros(pad, f)])
    # [e] -> [n, p, c] -> [p, n*3+c], edge id e = n*128 + p
    packed = np.stack([row_f.astype(f), col_f.astype(f), ew_f], axis=1)
    edges = np.ascontiguousarray(packed.reshape(ETILES, P, 3).transpose(1, 0, 2).reshape(P, ETILES * 3))

    c = np.ascontiguousarray
    return {
        "x_n": x,
        "x_t": c(x.T),
        "w0_n": W0,
        "w0_t": c(W0.T),
        "wih_t": c(w_ih.T),
        "whh_t": c(w_hh.T),
        "linw_t": c(lin_w.T),
        "bih_r": b_ih[None, :].copy(),
        "bhh_r": b_hh[None, :].copy(),
        "linb_r": lin_b[None, :].copy(),
        "cbias_c": conv_bias[:, None].copy(),
        "p_c": pool_p[:, None].copy(),
        "iota_t": np.tile(np.arange(N, dtype=f)[None, :], (P, 1)),
        "iota_c": np.arange(N, dtype=f)[:, None].copy(),
        "ones_c": np.ones((N, 1), f),
        "ones_r": np.ones((1, P), f),
        "edges": edges,
    }


def run(inputs, trace=False, n_cores=8):
    from concourse.bass_utils import run_bass_kernel_spmd

    if "nc" not in _CACHE:
        _CACHE["nc"] = _build()
    nc = _CACHE["nc"]
    im = _pack(inputs)
    res = run_bass_kernel_spmd(
        nc, [dict(im) for _ in range(n_cores)], list(range(n_cores)), trace=trace
    )
    out = np.asarray(res.results[0]["out"])
    return out, res


def kernel(**inputs) -> np.ndarray:
    out, _ = run(inputs, trace=False)
    return out
